# revision 32
# baseline (speedup 1.0000x reference)
"""Trainium2 Bass kernel for nn_AutoregressiveDecoder (gnn_message_passing).

reference math (N=512, D=256, H=64):
    x = z @ z.T
    M[i,r] = r < i;  colsum = (M @ adj) * M;  degs = max(colsum,1)^-0.5
    base = z @ W1[:256]          (W1[-1] helper row provably dead)
    per i:  d_i = M[i] * degs[i]
            Y_i   = adj @ (d_i * base)       [N,H]
            s_i   = (d_i * relu(Y_i)) @ W2   [N]
            t_i   = d_i * s_i
            S[i]  = d_i * (adj @ t_i)        [N]
    out = x + 0.5*(S + S.T)

v3 design notes:
  - |W2| is folded into W1 host-side (relu(y*|w|)=|w|*relu(y)); columns are
    permuted so the h-reduction handles signs with an add/sub split at tree
    level 0.  The per-element W2 multiply stage disappears.
  - d is kept in a duplicated (i,2) layout so the V = d (x) base build hits
    the DVE 2x fast path (all operands 2-byte, packed inner dims).
  - degs^-1/2 via one Abs_reciprocal_sqrt activation; together with Relu it
    lives in act table set 15 -> a single ACT_TABLE_LOAD, warmed at t=0.
  - x stays in PSUM until the final add (no copy-out).
  - i axis sharded over 8 cores in interleaved chunks of 16 so triangular
    bounds balance: every core sees bounds {128,256,384,512}.
"""
import sys

sys.path.insert(0, "/opt/trn_rl_repo")

import numpy as np
import ml_dtypes

N = 512
D = 256
H = 64
NCORES = 8
NI = 16
NCHUNKS = N // NI  # 32
CPC = NCHUNKS // NCORES  # 4
P = 128
KT = N // P  # 4
KD = D // P  # 2
BF = ml_dtypes.bfloat16

_cache = {}


def _chunks_of_core(k):
    return [k + NCORES * g for g in range(CPC)]


def _iset_of_core(k):
    out = []
    for c in _chunks_of_core(k):
        out.extend(range(NI * c, NI * (c + 1)))
    return np.array(out, dtype=np.int64)


def _build(n_add, fsign):
    """n_add: level-0 pairs (j, 32+j) with j < n_add are ADD, rest SUB.
    fsign: +-1.0 applied to the reduced sum (folded into the t multiply)."""
    import concourse.bacc as bacc
    import concourse.mybir as mybir
    from concourse import tile

    fp32 = mybir.dt.float32
    bf16 = mybir.dt.bfloat16
    AT = mybir.AluOpType
    AF = mybir.ActivationFunctionType

    nc = bacc.Bacc("TRN2", target_bir_lowering=False, debug=False, num_devices=NCORES)

    adj_in = nc.dram_tensor("adjb", [N, N], bf16, kind="ExternalInput")
    zt_in = nc.dram_tensor("zTb", [D, N], bf16, kind="ExternalInput")
    w1_in = nc.dram_tensor("w1b", [D, H], bf16, kind="ExternalInput")
    sm_in = nc.dram_tensor("smb", [P, KT * H + P + KD * H], bf16, kind="ExternalInput")

    pout = nc.dram_tensor("pout", [H, N], bf16, kind="ExternalOutput")
    stout = nc.dram_tensor("stout", [N, H], bf16, kind="ExternalOutput")

    with tile.TileContext(nc) as tc:
        with (
            tc.tile_pool(name="const", bufs=1) as cpool,
            tc.tile_pool(name="work", bufs=2) as wpool,
            tc.tile_pool(name="psA", bufs=3, space="PSUM") as psA,
            tc.tile_pool(name="psW", bufs=1, space="PSUM") as psW,
        ):
            # ---- act-table warm (set 15: abs_reciprocal_sqrt + relu) ----
            scr = cpool.tile([P, 16], fp32, tag="scr")
            nc.vector.memset(scr[:, :], 1.0)
            scr2 = cpool.tile([P, 16], bf16, tag="scr2")
            nc.scalar.activation(out=scr2[:, :], in_=scr[:, :], func=AF.Abs_reciprocal_sqrt)

            # ---- DMAs spread across 4 rings ----
            G = cpool.tile([P, KT, N], bf16, tag="G")
            sm = cpool.tile([P, KT * H + P + KD * H], bf16, tag="sm")
            MTb = sm[:, 0 : KT * H].rearrange("p (kt i) -> p kt i", kt=KT)
            ident = sm[:, KT * H : KT * H + P]
            smik = sm[:, KT * H + P :].rearrange("p (kd i) -> p kd i", kd=KD)
            zT = cpool.tile([P, KD, N], bf16, tag="zT")
            W1c = cpool.tile([P, KD, H], bf16, tag="W1c")

            nc.gpsimd.dma_start(out=sm[:, 0 : KT * H], in_=sm_in[:, 0 : KT * H])
            nc.sync.dma_start(out=G[:, 0, :], in_=adj_in[0:P, :])
            nc.scalar.dma_start(out=G[:, 1, :], in_=adj_in[P : 2 * P, :])
            nc.sync.dma_start(out=G[:, 2, :], in_=adj_in[2 * P : 3 * P, :])
            nc.scalar.dma_start(out=G[:, 3, :], in_=adj_in[3 * P :, :])
            nc.gpsimd.dma_start(
                out=W1c[:, :, :], in_=w1_in.ap().rearrange("(kt p) h -> p kt h", p=P)
            )
            nc.gpsimd.dma_start(out=zT[:, 0, :], in_=zt_in[0:P, :])
            nc.gpsimd.dma_start(out=zT[:, 1, :], in_=zt_in[P:, :])
            nc.sync.dma_start(out=sm[:, KT * H :], in_=sm_in[:, KT * H :])

            identf = cpool.tile([P, P], fp32, tag="identf")
            nc.gpsimd.tensor_copy(out=identf[:, :], in_=ident)

            # ---- Tb cleared once ----
            Tb = cpool.tile([P, KT, H], bf16, tag="Tb")
            nc.gpsimd.memset(Tb[:, :, :], 0.0)

            # ---- colsum (kt-outer so kt0 starts as soon as G0+mask land) ----
            mxt = cpool.tile([P, KT, H], fp32, tag="mxt")
            sq = cpool.tile([P, KT, H], bf16, tag="sq")
            # dTg: chunk-major dup layout [P, g, kt, (i,2)]
            dTg = cpool.tile([P, CPC, KT, 2 * NI], bf16, tag="dTg")
            for pb in range(KT):
                csps = psA.tile([P, NI * H], fp32, tag="psA", name="csps")
                ps = csps[:, 0:H]
                for kt in range(KT):
                    nc.tensor.matmul(
                        ps[:, :],
                        G[:, kt, pb * P : (pb + 1) * P],
                        MTb[:, kt, :],
                        start=(kt == 0),
                        stop=(kt == KT - 1),
                    )
                nc.vector.tensor_scalar_max(
                    out=mxt[:, pb, :], in0=ps[:, :], scalar1=1.0
                )
                nc.scalar.activation(
                    out=sq[:, pb, :], in_=mxt[:, pb, :], func=AF.Abs_reciprocal_sqrt
                )
                nc.vector.tensor_tensor(
                    out=dTg[:, :, pb, :].rearrange("p g (i two) -> p g i two", two=2),
                    in0=sq[:, pb, :]
                    .rearrange("p (g i) -> p g i", g=CPC)
                    .unsqueeze(3)
                    .broadcast_to((P, CPC, NI, 2)),
                    in1=MTb[:, pb, :]
                    .rearrange("p (g i) -> p g i", g=CPC)
                    .unsqueeze(3)
                    .broadcast_to((P, CPC, NI, 2)),
                    op=AT.mult,
                )
            dT2g = cpool.tile([P, CPC, KT, 2 * NI], bf16, tag="dT2g")

            # ---- base'' = z @ W1'' (bf16 out via ACT copy) ----
            bsb = cpool.tile([P, KT, H], bf16, tag="bsb")
            for pb in range(KT):
                bsps = psA.tile([P, NI * H], fp32, tag="psA", name="bsps")
                ps = bsps[:, 0:H]
                for kt in range(KD):
                    nc.tensor.matmul(
                        ps[:, :],
                        zT[:, kt, pb * P : (pb + 1) * P],
                        W1c[:, kt, :],
                        start=(kt == 0),
                        stop=(kt == KD - 1),
                    )
                nc.vector.tensor_copy(out=bsb[:, pb, :], in_=ps[:, :])

            # ---- main loop: V builds first (DVE leads), ascending conv ----
            xps = psW.tile([H, N], fp32, tag="xps")
            n_sub = 32 - n_add
            Vs = []
            for g in range(CPC):
                kts = g + 1
                V = cpool.tile([P, kts, NI, H], bf16, tag=f"V{g}", name=f"V{g}")
                nc.vector.tensor_tensor(
                    out=V[:, :, :, :].rearrange("p k i (w two) -> p k i w two", two=2),
                    in0=bsb[:, 0:kts, :]
                    .unsqueeze(2)
                    .broadcast_to((P, kts, NI, H))
                    .rearrange("p k i (w two) -> p k i w two", two=2),
                    in1=dTg[:, g, 0:kts, :]
                    .rearrange("p k (i two) -> p k i two", two=2)
                    .unsqueeze(3)
                    .broadcast_to((P, kts, NI, 32, 2)),
                    op=AT.mult,
                )
                Vs.append(V)
            nc.vector.tensor_tensor(
                out=dT2g[:, :, :, :],
                in0=dTg[:, :, :, :],
                in1=dTg[:, :, :, :],
                op=AT.mult,
            )
            for g in range(CPC):
                kts = g + 1
                if g == 1:
                    for kt in range(KD):
                        nc.tensor.matmul(
                            xps[:, :],
                            smik[:, kt, :],
                            zT[:, kt, :],
                            start=(kt == 0),
                            stop=(kt == KD - 1),
                        )
                Vf = Vs[g][:, :, :, :].rearrange("p k i h -> p k (i h)")
                RW = wpool.tile([P, kts, NI, H], bf16, tag="RW")
                for pb in range(kts):
                    yp = psA.tile([P, NI * H], fp32, tag="psA")
                    for kt in range(kts):
                        for cc in range(2):
                            nc.tensor.matmul(
                                yp[:, cc * 512 : (cc + 1) * 512],
                                G[:, kt, pb * P : (pb + 1) * P],
                                Vf[:, kt, cc * 512 : (cc + 1) * 512],
                                start=(kt == 0),
                                stop=(kt == kts - 1),
                            )
                    nc.scalar.activation(
                        out=RW[:, pb, :, :].rearrange("p i h -> p (i h)"),
                        in_=yp[:, :],
                        func=AF.Relu,
                    )
                # reduction: L0 add/sub split, L1, L2, reduce over last 8
                bufA = wpool.tile([P, kts, NI, 32], bf16, tag="bufA")
                bufB = wpool.tile([P, kts, NI, 16], bf16, tag="bufB")
                r3 = RW[:, :, :, :].rearrange("p k i h -> p (k i) h")
                a3 = bufA[:, :, :, :].rearrange("p k i h -> p (k i) h")
                b3 = bufB[:, :, :, :].rearrange("p k i h -> p (k i) h")
                if n_add > 0:
                    nc.vector.tensor_tensor(
                        out=a3[:, :, 0:n_add],
                        in0=r3[:, :, 0:n_add],
                        in1=r3[:, :, 32 : 32 + n_add],
                        op=AT.add,
                    )
                if n_sub > 0:
                    nc.vector.tensor_tensor(
                        out=a3[:, :, n_add:32],
                        in0=r3[:, :, n_add:32],
                        in1=r3[:, :, 32 + n_add : 64],
                        op=AT.subtract,
                    )
                nc.vector.tensor_tensor(
                    out=b3[:, :, :],
                    in0=a3[:, :, 0:16],
                    in1=a3[:, :, 16:32],
                    op=AT.add,
                )
                nc.vector.tensor_tensor(
                    out=a3[:, :, 0:8],
                    in0=b3[:, :, 0:8],
                    in1=b3[:, :, 8:16],
                    op=AT.add,
                )
                sres = wpool.tile([P, kts, NI], fp32, tag="sres")
                nc.vector.tensor_reduce(
                    out=sres[:, :, :].rearrange("p k i -> p (k i)").unsqueeze(2),
                    in_=a3[:, :, 0:8],
                    axis=mybir.AxisListType.X,
                    op=AT.add,
                )
                nc.vector.scalar_tensor_tensor(
                    out=Tb[:, 0:kts, NI * g : NI * (g + 1)],
                    in0=sres[:, :, :],
                    scalar=fsign,
                    in1=dT2g[:, g, 0:kts, :].rearrange(
                        "p k (i two) -> p k i two", two=2
                    )[:, :, :, 0],
                    op0=AT.mult,
                    op1=AT.mult,
                )

            # ---- 2nd conv + ST (0.5 folded) ----
            STf = cpool.tile([P, KT, H], fp32, tag="STf")
            STb = cpool.tile([P, KT, H], bf16, tag="STb")
            stoutv = stout.ap().rearrange("(pb p) i -> p pb i", p=P)
            for pb in range(KT):
                ops_t = psA.tile([P, NI * H], fp32, tag="psA", name="ops")
                ops = ops_t[:, 0:H]
                for kt in range(KT):
                    nc.tensor.matmul(
                        ops[:, :],
                        G[:, kt, pb * P : (pb + 1) * P],
                        Tb[:, kt, :],
                        start=(kt == 0),
                        stop=(kt == KT - 1),
                    )
                nc.vector.scalar_tensor_tensor(
                    out=STf[:, pb, :].rearrange("p (g i) -> p g i", g=CPC),
                    in0=ops[:, :].rearrange("p (g i) -> p g i", g=CPC),
                    scalar=0.5,
                    in1=dTg[:, :, pb, :].rearrange(
                        "p g (i two) -> p g i two", two=2
                    )[:, :, :, 0],
                    op0=AT.mult,
                    op1=AT.mult,
                )
                nc.gpsimd.tensor_copy(out=STb[:, pb, :], in_=STf[:, pb, :])
                nc.gpsimd.dma_start(out=stoutv[:, pb, :], in_=STb[:, pb, :])
            # ---- S rows transposed straight onto x in PSUM ----
            po = cpool.tile([H, N], bf16, tag="po")
            for pb in range(KT):
                nc.tensor.matmul(
                    xps[:, pb * P : (pb + 1) * P],
                    STf[:, pb, :],
                    identf[:, :],
                    is_transpose=True,
                    start=False,
                    stop=True,
                    skip_group_check=True,
                )
                if pb == 1:
                    nc.vector.tensor_copy(
                        out=po[:, 0 : 2 * P], in_=xps[:, 0 : 2 * P]
                    )
                    nc.scalar.dma_start(
                        out=pout[:, 0 : 2 * P], in_=po[:, 0 : 2 * P]
                    )
            nc.vector.tensor_copy(out=po[:, 2 * P :], in_=xps[:, 2 * P :])
            nc.scalar.dma_start(out=pout[:, 2 * P :], in_=po[:, 2 * P :])

    nc.compile()
    return nc


def _layout(W2):
    sig = W2[:, 0].astype(np.float64)
    pos = np.where(sig > 0)[0]
    neg = np.where(sig <= 0)[0]
    if len(pos) >= 32:
        lo, hi_main, hi_rest = pos[:32], pos[32:], neg
        fsign = 1.0
    else:
        lo, hi_main, hi_rest = neg[:32], neg[32:], pos
        fsign = -1.0
    n_add = len(hi_main)
    perm = np.concatenate([lo, hi_main, hi_rest])
    assert len(perm) == H and n_add <= 32
    return perm, int(n_add), float(fsign)


def _get_nc(n_add, fsign):
    key = ("nc", n_add, fsign)
    if key not in _cache:
        _cache[key] = _build(n_add, fsign)
    return _cache[key]


def _prepare_in_maps(z, adj, W1, W2):
    z = np.asarray(z, dtype=np.float32)
    adj = np.asarray(adj, dtype=np.float32)
    W1 = np.asarray(W1, dtype=np.float32)
    W2 = np.asarray(W2, dtype=np.float32)

    perm, n_add, fsign = _layout(W2)
    adjb = adj.astype(BF)
    zTb = np.ascontiguousarray(z.T).astype(BF)
    w1b = (W1[:D][:, perm] * np.abs(W2[perm, 0])[None, :]).astype(BF)
    ident = np.eye(P, dtype=np.float32)

    idx = np.arange(N)
    in_maps = []
    for k in range(NCORES):
        iset = _iset_of_core(k)
        MT = (idx[:, None] < iset[None, :]).astype(np.float32)  # [N, 64]
        MT_fold = MT.reshape(KT, P, H).transpose(1, 0, 2).reshape(P, KT * H)
        zik = (
            np.ascontiguousarray(z.T[:, iset])
            .reshape(KD, P, H)
            .transpose(1, 0, 2)
            .reshape(P, KD * H)
        )
        smb = np.concatenate([MT_fold, ident, zik], axis=1).astype(BF)
        in_maps.append({"adjb": adjb, "zTb": zTb, "w1b": w1b, "smb": smb})
    return in_maps, n_add, fsign


def kernel(z, adj, W1, W2):
    from concourse import bass_utils

    in_maps, n_add, fsign = _prepare_in_maps(z, adj, W1, W2)
    nc = _get_nc(n_add, fsign)
    res = bass_utils.run_bass_kernel_spmd(
        nc, in_maps, core_ids=list(range(NCORES)), trace=False
    )
    out = np.empty((N, N), dtype=np.float32)
    stf = np.empty((N, N), dtype=np.float32)
    for k in range(NCORES):
        iset = _iset_of_core(k)
        out[iset, :] = np.asarray(res.results[k]["pout"], dtype=np.float32)
        stf[:, iset] = np.asarray(res.results[k]["stout"], dtype=np.float32)
    out += stf  # 0.5 already folded into STf
    return out


# revision 33
# speedup vs baseline: 1.0359x; 1.0359x over previous
"""Trainium2 Bass kernel for nn_AutoregressiveDecoder (gnn_message_passing).

reference math (N=512, D=256, H=64):
    x = z @ z.T
    M[i,r] = r < i;  colsum = (M @ adj) * M;  degs = max(colsum,1)^-0.5
    base = z @ W1[:256]          (W1[-1] helper row provably dead)
    per i:  d_i = M[i] * degs[i]
            Y_i   = adj @ (d_i * base)       [N,H]
            s_i   = (d_i * relu(Y_i)) @ W2   [N]
            t_i   = d_i * s_i
            S[i]  = d_i * (adj @ t_i)        [N]
    out = x + 0.5*(S + S.T)

v3 design notes:
  - |W2| is folded into W1 host-side (relu(y*|w|)=|w|*relu(y)); columns are
    permuted so the h-reduction handles signs with an add/sub split at tree
    level 0.  The per-element W2 multiply stage disappears.
  - d is kept in a duplicated (i,2) layout so the V = d (x) base build hits
    the DVE 2x fast path (all operands 2-byte, packed inner dims).
  - degs^-1/2 via one Abs_reciprocal_sqrt activation; together with Relu it
    lives in act table set 15 -> a single ACT_TABLE_LOAD, warmed at t=0.
  - x stays in PSUM until the final add (no copy-out).
  - i axis sharded over 8 cores in interleaved chunks of 16 so triangular
    bounds balance: every core sees bounds {128,256,384,512}.
"""
import sys

sys.path.insert(0, "/opt/trn_rl_repo")

import numpy as np
import ml_dtypes

N = 512
D = 256
H = 64
NCORES = 8
NI = 16
NCHUNKS = N // NI  # 32
CPC = NCHUNKS // NCORES  # 4
P = 128
KT = N // P  # 4
KD = D // P  # 2
BF = ml_dtypes.bfloat16

_cache = {}


def _chunks_of_core(k):
    return [k + NCORES * g for g in range(CPC)]


def _iset_of_core(k):
    out = []
    for c in _chunks_of_core(k):
        out.extend(range(NI * c, NI * (c + 1)))
    return np.array(out, dtype=np.int64)


def _build(n_add, fsign):
    """n_add: level-0 pairs (j, 32+j) with j < n_add are ADD, rest SUB.
    fsign: +-1.0 applied to the reduced sum (folded into the t multiply)."""
    import concourse.bacc as bacc
    import concourse.mybir as mybir
    from concourse import tile

    fp32 = mybir.dt.float32
    bf16 = mybir.dt.bfloat16
    AT = mybir.AluOpType
    AF = mybir.ActivationFunctionType

    nc = bacc.Bacc("TRN2", target_bir_lowering=False, debug=False, num_devices=NCORES)

    adj_in = nc.dram_tensor("adjb", [N, N], bf16, kind="ExternalInput")
    zt_in = nc.dram_tensor("zTb", [D, N], bf16, kind="ExternalInput")
    w1_in = nc.dram_tensor("w1b", [D, H], bf16, kind="ExternalInput")
    sm_in = nc.dram_tensor("smb", [P, KT * H + P + KD * H], bf16, kind="ExternalInput")

    pout = nc.dram_tensor("pout", [H, N], bf16, kind="ExternalOutput")
    stout = nc.dram_tensor("stout", [N, H], fp32, kind="ExternalOutput")

    with tile.TileContext(nc) as tc:
        with (
            tc.tile_pool(name="const", bufs=1) as cpool,
            tc.tile_pool(name="work", bufs=2) as wpool,
            tc.tile_pool(name="psA", bufs=3, space="PSUM") as psA,
            tc.tile_pool(name="psW", bufs=1, space="PSUM") as psW,
        ):
            # ---- act-table warm (set 15: abs_reciprocal_sqrt + relu) ----
            scr = cpool.tile([P, 16], fp32, tag="scr")
            nc.vector.memset(scr[:, :], 1.0)
            scr2 = cpool.tile([P, 16], bf16, tag="scr2")
            nc.scalar.activation(out=scr2[:, :], in_=scr[:, :], func=AF.Abs_reciprocal_sqrt)

            # ---- DMAs spread across 4 rings ----
            G = cpool.tile([P, KT, N], bf16, tag="G")
            sm = cpool.tile([P, KT * H + P + KD * H], bf16, tag="sm")
            MTb = sm[:, 0 : KT * H].rearrange("p (kt i) -> p kt i", kt=KT)
            ident = sm[:, KT * H : KT * H + P]
            smik = sm[:, KT * H + P :].rearrange("p (kd i) -> p kd i", kd=KD)
            zT = cpool.tile([P, KD, N], bf16, tag="zT")
            W1c = cpool.tile([P, KD, H], bf16, tag="W1c")

            nc.gpsimd.dma_start(out=sm[:, 0 : KT * H], in_=sm_in[:, 0 : KT * H])
            nc.sync.dma_start(out=G[:, 0, :], in_=adj_in[0:P, :])
            nc.scalar.dma_start(out=G[:, 1, :], in_=adj_in[P : 2 * P, :])
            nc.sync.dma_start(out=G[:, 2, :], in_=adj_in[2 * P : 3 * P, :])
            nc.scalar.dma_start(out=G[:, 3, :], in_=adj_in[3 * P :, :])
            nc.gpsimd.dma_start(
                out=W1c[:, :, :], in_=w1_in.ap().rearrange("(kt p) h -> p kt h", p=P)
            )
            nc.gpsimd.dma_start(out=zT[:, 0, :], in_=zt_in[0:P, :])
            nc.gpsimd.dma_start(out=zT[:, 1, :], in_=zt_in[P:, :])
            nc.sync.dma_start(out=sm[:, KT * H :], in_=sm_in[:, KT * H :])

            identf = cpool.tile([P, P], fp32, tag="identf")
            nc.gpsimd.tensor_copy(out=identf[:, :], in_=ident)

            # ---- Tb cleared once ----
            Tb = cpool.tile([P, KT, H], bf16, tag="Tb")
            nc.gpsimd.memset(Tb[:, :, :], 0.0)

            # ---- colsum (kt-outer so kt0 starts as soon as G0+mask land) ----
            mxt = cpool.tile([P, KT, H], fp32, tag="mxt")
            sq = cpool.tile([P, KT, H], bf16, tag="sq")
            # dTg: chunk-major dup layout [P, g, kt, (i,2)]
            dTg = cpool.tile([P, CPC, KT, 2 * NI], bf16, tag="dTg")
            for pb in range(KT):
                csps = psA.tile([P, NI * H], fp32, tag="psA", name="csps")
                ps = csps[:, 0:H]
                for kt in range(KT):
                    nc.tensor.matmul(
                        ps[:, :],
                        G[:, kt, pb * P : (pb + 1) * P],
                        MTb[:, kt, :],
                        start=(kt == 0),
                        stop=(kt == KT - 1),
                    )
                nc.vector.tensor_scalar_max(
                    out=mxt[:, pb, :], in0=ps[:, :], scalar1=1.0
                )
                nc.scalar.activation(
                    out=sq[:, pb, :], in_=mxt[:, pb, :], func=AF.Abs_reciprocal_sqrt
                )
                nc.vector.tensor_tensor(
                    out=dTg[:, :, pb, :].rearrange("p g (i two) -> p g i two", two=2),
                    in0=sq[:, pb, :]
                    .rearrange("p (g i) -> p g i", g=CPC)
                    .unsqueeze(3)
                    .broadcast_to((P, CPC, NI, 2)),
                    in1=MTb[:, pb, :]
                    .rearrange("p (g i) -> p g i", g=CPC)
                    .unsqueeze(3)
                    .broadcast_to((P, CPC, NI, 2)),
                    op=AT.mult,
                )
            dT2g = cpool.tile([P, CPC, KT, 2 * NI], bf16, tag="dT2g")

            # ---- base'' = z @ W1'' (bf16 out via ACT copy) ----
            bsb = cpool.tile([P, KT, H], bf16, tag="bsb")
            for pb in range(KT):
                bsps = psA.tile([P, NI * H], fp32, tag="psA", name="bsps")
                ps = bsps[:, 0:H]
                for kt in range(KD):
                    nc.tensor.matmul(
                        ps[:, :],
                        zT[:, kt, pb * P : (pb + 1) * P],
                        W1c[:, kt, :],
                        start=(kt == 0),
                        stop=(kt == KD - 1),
                    )
                nc.vector.tensor_copy(out=bsb[:, pb, :], in_=ps[:, :])

            # ---- main loop: V builds first (DVE leads), ascending conv ----
            xps = psW.tile([H, N], fp32, tag="xps")
            n_sub = 32 - n_add
            Vs = []
            for g in range(CPC):
                kts = g + 1
                V = cpool.tile([P, kts, NI, H], bf16, tag=f"V{g}", name=f"V{g}")
                nc.vector.tensor_tensor(
                    out=V[:, :, :, :].rearrange("p k i (w two) -> p k i w two", two=2),
                    in0=bsb[:, 0:kts, :]
                    .unsqueeze(2)
                    .broadcast_to((P, kts, NI, H))
                    .rearrange("p k i (w two) -> p k i w two", two=2),
                    in1=dTg[:, g, 0:kts, :]
                    .rearrange("p k (i two) -> p k i two", two=2)
                    .unsqueeze(3)
                    .broadcast_to((P, kts, NI, 32, 2)),
                    op=AT.mult,
                )
                Vs.append(V)
            nc.vector.tensor_tensor(
                out=dT2g[:, :, :, :],
                in0=dTg[:, :, :, :],
                in1=dTg[:, :, :, :],
                op=AT.mult,
            )
            for g in range(CPC):
                kts = g + 1
                if g == 1:
                    for kt in range(KD):
                        nc.tensor.matmul(
                            xps[:, :],
                            smik[:, kt, :],
                            zT[:, kt, :],
                            start=(kt == 0),
                            stop=(kt == KD - 1),
                        )
                Vf = Vs[g][:, :, :, :].rearrange("p k i h -> p k (i h)")
                RW = wpool.tile([P, kts, NI, H], bf16, tag="RW")
                for pb in range(kts):
                    yp = psA.tile([P, NI * H], fp32, tag="psA")
                    for kt in range(kts):
                        for cc in range(2):
                            nc.tensor.matmul(
                                yp[:, cc * 512 : (cc + 1) * 512],
                                G[:, kt, pb * P : (pb + 1) * P],
                                Vf[:, kt, cc * 512 : (cc + 1) * 512],
                                start=(kt == 0),
                                stop=(kt == kts - 1),
                            )
                    nc.scalar.activation(
                        out=RW[:, pb, :, :].rearrange("p i h -> p (i h)"),
                        in_=yp[:, :],
                        func=AF.Relu,
                    )
                # reduction: L0 add/sub split, L1, L2, reduce over last 8
                bufA = wpool.tile([P, kts, NI, 32], bf16, tag="bufA")
                bufB = wpool.tile([P, kts, NI, 16], bf16, tag="bufB")
                r3 = RW[:, :, :, :].rearrange("p k i h -> p (k i) h")
                a3 = bufA[:, :, :, :].rearrange("p k i h -> p (k i) h")
                b3 = bufB[:, :, :, :].rearrange("p k i h -> p (k i) h")
                if n_add > 0:
                    nc.vector.tensor_tensor(
                        out=a3[:, :, 0:n_add],
                        in0=r3[:, :, 0:n_add],
                        in1=r3[:, :, 32 : 32 + n_add],
                        op=AT.add,
                    )
                if n_sub > 0:
                    nc.vector.tensor_tensor(
                        out=a3[:, :, n_add:32],
                        in0=r3[:, :, n_add:32],
                        in1=r3[:, :, 32 + n_add : 64],
                        op=AT.subtract,
                    )
                nc.vector.tensor_tensor(
                    out=b3[:, :, :],
                    in0=a3[:, :, 0:16],
                    in1=a3[:, :, 16:32],
                    op=AT.add,
                )
                nc.vector.tensor_tensor(
                    out=a3[:, :, 0:8],
                    in0=b3[:, :, 0:8],
                    in1=b3[:, :, 8:16],
                    op=AT.add,
                )
                sres = wpool.tile([P, kts, NI], fp32, tag="sres")
                nc.vector.tensor_reduce(
                    out=sres[:, :, :].rearrange("p k i -> p (k i)").unsqueeze(2),
                    in_=a3[:, :, 0:8],
                    axis=mybir.AxisListType.X,
                    op=AT.add,
                )
                nc.vector.scalar_tensor_tensor(
                    out=Tb[:, 0:kts, NI * g : NI * (g + 1)],
                    in0=sres[:, :, :],
                    scalar=fsign,
                    in1=dT2g[:, g, 0:kts, :].rearrange(
                        "p k (i two) -> p k i two", two=2
                    )[:, :, :, 0],
                    op0=AT.mult,
                    op1=AT.mult,
                )

            # ---- 2nd conv + ST (0.5 folded) ----
            STf = cpool.tile([P, KT, H], fp32, tag="STf")
            for pb in range(KT):
                ops_t = psA.tile([P, NI * H], fp32, tag="psA", name="ops")
                ops = ops_t[:, 0:H]
                for kt in range(KT):
                    nc.tensor.matmul(
                        ops[:, :],
                        G[:, kt, pb * P : (pb + 1) * P],
                        Tb[:, kt, :],
                        start=(kt == 0),
                        stop=(kt == KT - 1),
                    )
                nc.vector.scalar_tensor_tensor(
                    out=STf[:, pb, :].rearrange("p (g i) -> p g i", g=CPC),
                    in0=ops[:, :].rearrange("p (g i) -> p g i", g=CPC),
                    scalar=0.5,
                    in1=dTg[:, :, pb, :].rearrange(
                        "p g (i two) -> p g i two", two=2
                    )[:, :, :, 0],
                    op0=AT.mult,
                    op1=AT.mult,
                )

            # ---- S rows transposed straight onto x in PSUM ----
            po = cpool.tile([H, N], bf16, tag="po")
            nc.sync.dma_start(
                out=stout.ap().rearrange("(pb p) i -> p pb i", p=P), in_=STf[:, :, :]
            )
            for pb in range(KT):
                nc.tensor.matmul(
                    xps[:, pb * P : (pb + 1) * P],
                    STf[:, pb, :],
                    identf[:, :],
                    is_transpose=True,
                    start=False,
                    stop=True,
                    skip_group_check=True,
                )
            nc.vector.tensor_copy(out=po[:, :], in_=xps[:, :])
            nc.scalar.dma_start(out=pout[:, :], in_=po[:, :])

    nc.compile()
    return nc


def _layout(W2):
    sig = W2[:, 0].astype(np.float64)
    pos = np.where(sig > 0)[0]
    neg = np.where(sig <= 0)[0]
    if len(pos) >= 32:
        lo, hi_main, hi_rest = pos[:32], pos[32:], neg
        fsign = 1.0
    else:
        lo, hi_main, hi_rest = neg[:32], neg[32:], pos
        fsign = -1.0
    n_add = len(hi_main)
    perm = np.concatenate([lo, hi_main, hi_rest])
    assert len(perm) == H and n_add <= 32
    return perm, int(n_add), float(fsign)


def _get_nc(n_add, fsign):
    key = ("nc", n_add, fsign)
    if key not in _cache:
        _cache[key] = _build(n_add, fsign)
    return _cache[key]


def _prepare_in_maps(z, adj, W1, W2):
    z = np.asarray(z, dtype=np.float32)
    adj = np.asarray(adj, dtype=np.float32)
    W1 = np.asarray(W1, dtype=np.float32)
    W2 = np.asarray(W2, dtype=np.float32)

    perm, n_add, fsign = _layout(W2)
    adjb = adj.astype(BF)
    zTb = np.ascontiguousarray(z.T).astype(BF)
    w1b = (W1[:D][:, perm] * np.abs(W2[perm, 0])[None, :]).astype(BF)
    ident = np.eye(P, dtype=np.float32)

    idx = np.arange(N)
    in_maps = []
    for k in range(NCORES):
        iset = _iset_of_core(k)
        MT = (idx[:, None] < iset[None, :]).astype(np.float32)  # [N, 64]
        MT_fold = MT.reshape(KT, P, H).transpose(1, 0, 2).reshape(P, KT * H)
        zik = (
            np.ascontiguousarray(z.T[:, iset])
            .reshape(KD, P, H)
            .transpose(1, 0, 2)
            .reshape(P, KD * H)
        )
        smb = np.concatenate([MT_fold, ident, zik], axis=1).astype(BF)
        in_maps.append({"adjb": adjb, "zTb": zTb, "w1b": w1b, "smb": smb})
    return in_maps, n_add, fsign


def kernel(z, adj, W1, W2):
    from concourse import bass_utils

    in_maps, n_add, fsign = _prepare_in_maps(z, adj, W1, W2)
    nc = _get_nc(n_add, fsign)
    res = bass_utils.run_bass_kernel_spmd(
        nc, in_maps, core_ids=list(range(NCORES)), trace=False
    )
    out = np.empty((N, N), dtype=np.float32)
    stf = np.empty((N, N), dtype=np.float32)
    for k in range(NCORES):
        iset = _iset_of_core(k)
        out[iset, :] = np.asarray(res.results[k]["pout"], dtype=np.float32)
        stf[:, iset] = np.asarray(res.results[k]["stout"], dtype=np.float32)
    out += stf  # 0.5 already folded into STf
    return out


# revision 34
# speedup vs baseline: 1.0701x; 1.0331x over previous
"""Trainium2 Bass kernel for nn_AutoregressiveDecoder (gnn_message_passing).

reference math (N=512, D=256, H=64):
    x = z @ z.T
    M[i,r] = r < i;  colsum = (M @ adj) * M;  degs = max(colsum,1)^-0.5
    base = z @ W1[:256]          (W1[-1] helper row provably dead)
    per i:  d_i = M[i] * degs[i]
            Y_i   = adj @ (d_i * base)       [N,H]
            s_i   = (d_i * relu(Y_i)) @ W2   [N]
            t_i   = d_i * s_i
            S[i]  = d_i * (adj @ t_i)        [N]
    out = x + 0.5*(S + S.T)

v3 design notes:
  - |W2| is folded into W1 host-side (relu(y*|w|)=|w|*relu(y)); columns are
    permuted so the h-reduction handles signs with an add/sub split at tree
    level 0.  The per-element W2 multiply stage disappears.
  - d is kept in a duplicated (i,2) layout so the V = d (x) base build hits
    the DVE 2x fast path (all operands 2-byte, packed inner dims).
  - degs^-1/2 via one Abs_reciprocal_sqrt activation; together with Relu it
    lives in act table set 15 -> a single ACT_TABLE_LOAD, warmed at t=0.
  - x stays in PSUM until the final add (no copy-out).
  - i axis sharded over 8 cores in interleaved chunks of 16 so triangular
    bounds balance: every core sees bounds {128,256,384,512}.
"""
import sys

sys.path.insert(0, "/opt/trn_rl_repo")

import numpy as np
import ml_dtypes

N = 512
D = 256
H = 64
NCORES = 8
NI = 16
NCHUNKS = N // NI  # 32
CPC = NCHUNKS // NCORES  # 4
P = 128
KT = N // P  # 4
KD = D // P  # 2
BF = ml_dtypes.bfloat16

_cache = {}


def _chunks_of_core(k):
    return [k + NCORES * g for g in range(CPC)]


def _iset_of_core(k):
    out = []
    for c in _chunks_of_core(k):
        out.extend(range(NI * c, NI * (c + 1)))
    return np.array(out, dtype=np.int64)


def _build(n_add, fsign):
    """n_add: level-0 pairs (j, 32+j) with j < n_add are ADD, rest SUB.
    fsign: +-1.0 applied to the reduced sum (folded into the t multiply)."""
    import concourse.bacc as bacc
    import concourse.mybir as mybir
    from concourse import tile

    fp32 = mybir.dt.float32
    bf16 = mybir.dt.bfloat16
    AT = mybir.AluOpType
    AF = mybir.ActivationFunctionType

    nc = bacc.Bacc("TRN2", target_bir_lowering=False, debug=False, num_devices=NCORES)

    adj_in = nc.dram_tensor("adjb", [N, N], bf16, kind="ExternalInput")
    zt_in = nc.dram_tensor("zTb", [D, N], bf16, kind="ExternalInput")
    w1_in = nc.dram_tensor("w1b", [D, H], bf16, kind="ExternalInput")
    sm_in = nc.dram_tensor("smb", [P, KT * H + P + KD * H], bf16, kind="ExternalInput")

    pout = nc.dram_tensor("pout", [H, N], bf16, kind="ExternalOutput")
    stout = nc.dram_tensor("stout", [N, H], fp32, kind="ExternalOutput")

    with tile.TileContext(nc) as tc:
        with (
            tc.tile_pool(name="const", bufs=1) as cpool,
            tc.tile_pool(name="work", bufs=2) as wpool,
            tc.tile_pool(name="psA", bufs=3, space="PSUM") as psA,
            tc.tile_pool(name="psW", bufs=1, space="PSUM") as psW,
        ):
            # ---- act-table warm (set 15: abs_reciprocal_sqrt + relu) ----
            scr = cpool.tile([P, 16], fp32, tag="scr")
            nc.vector.memset(scr[:, :], 1.0)
            scr2 = cpool.tile([P, 16], bf16, tag="scr2")
            nc.scalar.activation(out=scr2[:, :], in_=scr[:, :], func=AF.Abs_reciprocal_sqrt)

            # ---- DMAs spread across 4 rings ----
            G = cpool.tile([P, KT, N], bf16, tag="G")
            sm = cpool.tile([P, KT * H + P + KD * H], bf16, tag="sm")
            MTb = sm[:, 0 : KT * H].rearrange("p (kt i) -> p kt i", kt=KT)
            ident = sm[:, KT * H : KT * H + P]
            smik = sm[:, KT * H + P :].rearrange("p (kd i) -> p kd i", kd=KD)
            zT = cpool.tile([P, KD, N], bf16, tag="zT")
            W1c = cpool.tile([P, KD, H], bf16, tag="W1c")

            nc.gpsimd.dma_start(out=sm[:, 0 : KT * H], in_=sm_in[:, 0 : KT * H])
            nc.sync.dma_start(out=G[:, 0, :], in_=adj_in[0:P, :])
            nc.scalar.dma_start(out=G[:, 1, :], in_=adj_in[P : 2 * P, :])
            nc.sync.dma_start(out=G[:, 2, :], in_=adj_in[2 * P : 3 * P, :])
            nc.scalar.dma_start(out=G[:, 3, :], in_=adj_in[3 * P :, :])
            nc.gpsimd.dma_start(
                out=W1c[:, :, :], in_=w1_in.ap().rearrange("(kt p) h -> p kt h", p=P)
            )
            nc.gpsimd.dma_start(out=zT[:, 0, :], in_=zt_in[0:P, :])
            nc.gpsimd.dma_start(out=zT[:, 1, :], in_=zt_in[P:, :])
            nc.sync.dma_start(out=sm[:, KT * H :], in_=sm_in[:, KT * H :])

            identf = cpool.tile([P, P], fp32, tag="identf")
            nc.gpsimd.tensor_copy(out=identf[:, :], in_=ident)

            # ---- Tb cleared once ----
            Tb = cpool.tile([P, KT, H], bf16, tag="Tb")
            nc.gpsimd.memset(Tb[:, :, :], 0.0)

            # ---- colsum (kt-outer so kt0 starts as soon as G0+mask land) ----
            mxt = cpool.tile([P, KT, H], fp32, tag="mxt")
            sq = cpool.tile([P, KT, H], bf16, tag="sq")
            # dTg: chunk-major dup layout [P, g, kt, (i,2)]
            dTg = cpool.tile([P, CPC, KT, 2 * NI], bf16, tag="dTg")
            for pb in range(KT):
                csps = psA.tile([P, NI * H], fp32, tag="psA", name="csps")
                ps = csps[:, 0:H]
                for kt in range(KT):
                    nc.tensor.matmul(
                        ps[:, :],
                        G[:, kt, pb * P : (pb + 1) * P],
                        MTb[:, kt, :],
                        start=(kt == 0),
                        stop=(kt == KT - 1),
                    )
                nc.vector.tensor_scalar_max(
                    out=mxt[:, pb, :], in0=ps[:, :], scalar1=1.0
                )
                nc.scalar.activation(
                    out=sq[:, pb, :], in_=mxt[:, pb, :], func=AF.Abs_reciprocal_sqrt
                )
                nc.vector.tensor_tensor(
                    out=dTg[:, :, pb, :].rearrange("p g (i two) -> p g i two", two=2),
                    in0=sq[:, pb, :]
                    .rearrange("p (g i) -> p g i", g=CPC)
                    .unsqueeze(3)
                    .broadcast_to((P, CPC, NI, 2)),
                    in1=MTb[:, pb, :]
                    .rearrange("p (g i) -> p g i", g=CPC)
                    .unsqueeze(3)
                    .broadcast_to((P, CPC, NI, 2)),
                    op=AT.mult,
                )
            dT2g = cpool.tile([P, CPC, KT, 2 * NI], bf16, tag="dT2g")

            # ---- base'' = z @ W1'' (bf16 out via ACT copy) ----
            bsb = cpool.tile([P, KT, H], bf16, tag="bsb")
            for pb in range(KT):
                bsps = psA.tile([P, NI * H], fp32, tag="psA", name="bsps")
                ps = bsps[:, 0:H]
                for kt in range(KD):
                    nc.tensor.matmul(
                        ps[:, :],
                        zT[:, kt, pb * P : (pb + 1) * P],
                        W1c[:, kt, :],
                        start=(kt == 0),
                        stop=(kt == KD - 1),
                    )
                nc.scalar.activation(
                    out=bsb[:, pb, :], in_=ps[:, :], func=AF.Identity
                )

            # ---- main loop: V builds first (DVE leads), ascending conv ----
            xps = psW.tile([H, N], fp32, tag="xps")
            n_sub = 32 - n_add
            Vs = []
            for g in range(CPC):
                kts = g + 1
                V = cpool.tile([P, kts, NI, H], bf16, tag=f"V{g}", name=f"V{g}")
                nc.vector.tensor_tensor(
                    out=V[:, :, :, :].rearrange("p k i (w two) -> p k i w two", two=2),
                    in0=bsb[:, 0:kts, :]
                    .unsqueeze(2)
                    .broadcast_to((P, kts, NI, H))
                    .rearrange("p k i (w two) -> p k i w two", two=2),
                    in1=dTg[:, g, 0:kts, :]
                    .rearrange("p k (i two) -> p k i two", two=2)
                    .unsqueeze(3)
                    .broadcast_to((P, kts, NI, 32, 2)),
                    op=AT.mult,
                )
                Vs.append(V)
            nc.vector.tensor_tensor(
                out=dT2g[:, :, :, :],
                in0=dTg[:, :, :, :],
                in1=dTg[:, :, :, :],
                op=AT.mult,
            )
            for g in range(CPC):
                kts = g + 1
                if g == 1:
                    for kt in range(KD):
                        nc.tensor.matmul(
                            xps[:, :],
                            smik[:, kt, :],
                            zT[:, kt, :],
                            start=(kt == 0),
                            stop=(kt == KD - 1),
                        )
                Vf = Vs[g][:, :, :, :].rearrange("p k i h -> p k (i h)")
                RW = wpool.tile([P, kts, NI, H], bf16, tag="RW")
                for pb in range(kts):
                    yp = psA.tile([P, NI * H], fp32, tag="psA")
                    for kt in range(kts):
                        for cc in range(2):
                            nc.tensor.matmul(
                                yp[:, cc * 512 : (cc + 1) * 512],
                                G[:, kt, pb * P : (pb + 1) * P],
                                Vf[:, kt, cc * 512 : (cc + 1) * 512],
                                start=(kt == 0),
                                stop=(kt == kts - 1),
                            )
                    nc.scalar.activation(
                        out=RW[:, pb, :, :].rearrange("p i h -> p (i h)"),
                        in_=yp[:, :],
                        func=AF.Relu,
                    )
                # reduction: L0 add/sub split, L1, L2, reduce over last 8
                bufA = wpool.tile([P, kts, NI, 32], bf16, tag="bufA")
                bufB = wpool.tile([P, kts, NI, 16], bf16, tag="bufB")
                r3 = RW[:, :, :, :].rearrange("p k i h -> p (k i) h")
                a3 = bufA[:, :, :, :].rearrange("p k i h -> p (k i) h")
                b3 = bufB[:, :, :, :].rearrange("p k i h -> p (k i) h")
                if n_add > 0:
                    nc.vector.tensor_tensor(
                        out=a3[:, :, 0:n_add],
                        in0=r3[:, :, 0:n_add],
                        in1=r3[:, :, 32 : 32 + n_add],
                        op=AT.add,
                    )
                if n_sub > 0:
                    nc.vector.tensor_tensor(
                        out=a3[:, :, n_add:32],
                        in0=r3[:, :, n_add:32],
                        in1=r3[:, :, 32 + n_add : 64],
                        op=AT.subtract,
                    )
                nc.vector.tensor_tensor(
                    out=b3[:, :, :],
                    in0=a3[:, :, 0:16],
                    in1=a3[:, :, 16:32],
                    op=AT.add,
                )
                nc.vector.tensor_tensor(
                    out=a3[:, :, 0:8],
                    in0=b3[:, :, 0:8],
                    in1=b3[:, :, 8:16],
                    op=AT.add,
                )
                sres = wpool.tile([P, kts, NI], fp32, tag="sres")
                nc.vector.tensor_reduce(
                    out=sres[:, :, :].rearrange("p k i -> p (k i)").unsqueeze(2),
                    in_=a3[:, :, 0:8],
                    axis=mybir.AxisListType.X,
                    op=AT.add,
                )
                nc.vector.scalar_tensor_tensor(
                    out=Tb[:, 0:kts, NI * g : NI * (g + 1)],
                    in0=sres[:, :, :],
                    scalar=fsign,
                    in1=dT2g[:, g, 0:kts, :].rearrange(
                        "p k (i two) -> p k i two", two=2
                    )[:, :, :, 0],
                    op0=AT.mult,
                    op1=AT.mult,
                )

            # ---- 2nd conv + ST (0.5 folded) ----
            STf = cpool.tile([P, KT, H], fp32, tag="STf")
            for pb in range(KT):
                ops_t = psA.tile([P, NI * H], fp32, tag="psA", name="ops")
                ops = ops_t[:, 0:H]
                for kt in range(KT):
                    nc.tensor.matmul(
                        ops[:, :],
                        G[:, kt, pb * P : (pb + 1) * P],
                        Tb[:, kt, :],
                        start=(kt == 0),
                        stop=(kt == KT - 1),
                    )
                nc.vector.scalar_tensor_tensor(
                    out=STf[:, pb, :].rearrange("p (g i) -> p g i", g=CPC),
                    in0=ops[:, :].rearrange("p (g i) -> p g i", g=CPC),
                    scalar=0.5,
                    in1=dTg[:, :, pb, :].rearrange(
                        "p g (i two) -> p g i two", two=2
                    )[:, :, :, 0],
                    op0=AT.mult,
                    op1=AT.mult,
                )

            # ---- S rows transposed straight onto x in PSUM ----
            po = cpool.tile([H, N], bf16, tag="po")
            nc.sync.dma_start(
                out=stout.ap().rearrange("(pb p) i -> p pb i", p=P), in_=STf[:, :, :]
            )
            for pb in range(KT):
                nc.tensor.matmul(
                    xps[:, pb * P : (pb + 1) * P],
                    STf[:, pb, :],
                    identf[:, :],
                    is_transpose=True,
                    start=False,
                    stop=True,
                    skip_group_check=True,
                )
            nc.vector.tensor_copy(out=po[:, :], in_=xps[:, :])
            nc.scalar.dma_start(out=pout[:, :], in_=po[:, :])

    nc.compile()
    return nc


def _layout(W2):
    sig = W2[:, 0].astype(np.float64)
    pos = np.where(sig > 0)[0]
    neg = np.where(sig <= 0)[0]
    if len(pos) >= 32:
        lo, hi_main, hi_rest = pos[:32], pos[32:], neg
        fsign = 1.0
    else:
        lo, hi_main, hi_rest = neg[:32], neg[32:], pos
        fsign = -1.0
    n_add = len(hi_main)
    perm = np.concatenate([lo, hi_main, hi_rest])
    assert len(perm) == H and n_add <= 32
    return perm, int(n_add), float(fsign)


def _get_nc(n_add, fsign):
    key = ("nc", n_add, fsign)
    if key not in _cache:
        _cache[key] = _build(n_add, fsign)
    return _cache[key]


def _prepare_in_maps(z, adj, W1, W2):
    z = np.asarray(z, dtype=np.float32)
    adj = np.asarray(adj, dtype=np.float32)
    W1 = np.asarray(W1, dtype=np.float32)
    W2 = np.asarray(W2, dtype=np.float32)

    perm, n_add, fsign = _layout(W2)
    adjb = adj.astype(BF)
    zTb = np.ascontiguousarray(z.T).astype(BF)
    w1b = (W1[:D][:, perm] * np.abs(W2[perm, 0])[None, :]).astype(BF)
    ident = np.eye(P, dtype=np.float32)

    idx = np.arange(N)
    in_maps = []
    for k in range(NCORES):
        iset = _iset_of_core(k)
        MT = (idx[:, None] < iset[None, :]).astype(np.float32)  # [N, 64]
        MT_fold = MT.reshape(KT, P, H).transpose(1, 0, 2).reshape(P, KT * H)
        zik = (
            np.ascontiguousarray(z.T[:, iset])
            .reshape(KD, P, H)
            .transpose(1, 0, 2)
            .reshape(P, KD * H)
        )
        smb = np.concatenate([MT_fold, ident, zik], axis=1).astype(BF)
        in_maps.append({"adjb": adjb, "zTb": zTb, "w1b": w1b, "smb": smb})
    return in_maps, n_add, fsign


def kernel(z, adj, W1, W2):
    from concourse import bass_utils

    in_maps, n_add, fsign = _prepare_in_maps(z, adj, W1, W2)
    nc = _get_nc(n_add, fsign)
    res = bass_utils.run_bass_kernel_spmd(
        nc, in_maps, core_ids=list(range(NCORES)), trace=False
    )
    out = np.empty((N, N), dtype=np.float32)
    stf = np.empty((N, N), dtype=np.float32)
    for k in range(NCORES):
        iset = _iset_of_core(k)
        out[iset, :] = np.asarray(res.results[k]["pout"], dtype=np.float32)
        stf[:, iset] = np.asarray(res.results[k]["stout"], dtype=np.float32)
    out += stf  # 0.5 already folded into STf
    return out
